# revision 1
# baseline (speedup 1.0000x reference)
"""Trainium2 Bass kernel for nn_ConvFilter (geometric-series conv filter).

Math (per batch b, output position l, feature f):
    t[o,l]  = sum_{i,k} conv_w[o,i,k] * x[l+k,i]          (valid conv, L=S-K+1)
    tau     = sigmoid(t + bias)
    out     = (sum_i tau^(7-i) * x[l+i,f]) / (sum_i tau^i)

Implementation:
  * transposed layout [feature, seq] on device; host pre/post-transposes.
  * conv: 16 accumulating fp32r matmuls per 512-wide l-tile (full-rate PE);
    two overlapping l-tiles (0 and L-512) since fp32r needs even free sizes.
  * numerator in fp16 on DVE (2x packed mode) with powers from ACT Squares:
        q_j = tau*x_{2j} + x_{2j+1}
        N   = (q0*T2 + q1)*T4 + (q2*T2 + q3),   T2 = tau^2, T4 = tau^4
    odd-shift windows read a one-element-shifted fp16 copy of x so every
    window stays 4-byte aligned (keeps the DVE 2x mode).
  * denominator fp32: D = (1+tau)(1+tau^2)(1+tau^4) as one custom DVE op,
    reciprocal via reciprocal_approx_fast; out = N * r (fp32).
  * engine split: ACT does sigmoid/converts/squares, DVE the main chain,
    GPSIMD the independent side-branch, PE only matmuls.
  * data-parallel over batch: 8 batches/core on 8 cores, weights replicated.
"""

import numpy as np
from contextlib import ExitStack

import concourse.bass as bass
import concourse.tile as tile
from concourse import bacc, mybir
from concourse.bass_utils import run_bass_kernel_spmd
from concourse import dve_ops
from concourse.dve_ops import DveOp
from concourse.dve_spec import Spec, Src0, Src1, lower, sq, One, _has_src1
from concourse.dve_uop import DveOpSpec

B, S, F, K = 64, 1024, 256, 8
L = S - K + 1  # 1017
NCORES = 8
BPC = B // NCORES
P = 128
NFB = F // P  # 2 feature blocks
LT = 512      # matmul l-tile width (one PSUM bank)
LE = L + 1    # even fp16 elementwise width (DVE 2x mode needs even counts)


def _register_op(name, spec, subdim=False):
    for existing in dve_ops.OPS:
        if existing.name == name:
            return existing
    shas = {}
    for ver in ("v3", "v4"):
        tmp = DveOpSpec(name=name, opcode=0, uops=lower(spec, ver=ver),
                        rd1_en=_has_src1(spec))
        shas[ver] = tmp.sha(ver)
    op = DveOp(name, spec, subdim=subdim, uops_sha=shas)
    dve_ops.OPS.append(op)
    dve_ops.CUSTOM_DVE_SPECS[name] = spec
    dve_ops._SUB_OPCODE_FOR_NAME[name] = (
        dve_ops._CUSTOM_DVE_ROW_BASE + len(dve_ops.OPS) - 1
    )
    assert dve_ops._SUB_OPCODE_FOR_NAME[name] < 0x20
    return op


def _get_ops():
    _t2 = sq(Src0)
    _t4 = sq(_t2)
    denom_spec = Spec(
        body=(Src0 + One) * (_t2 + One) * (_t4 + One),
        reference=lambda in0, in1, s0, s1, imm2: (
            (1.0 + in0) * (1.0 + in0 * in0) * (1.0 + in0 ** 4)
        ).astype(np.float32),
    )
    return _register_op("ANT_CF_DENOM", denom_spec)


def build_module():
    DENOM_OP = _get_ops()
    f32 = mybir.dt.float32
    f32r = mybir.dt.float32r
    f16 = mybir.dt.float16
    TT = mybir.AluOpType
    SIG = mybir.ActivationFunctionType.Sigmoid
    SQU = mybir.ActivationFunctionType.Square
    CPY = mybir.ActivationFunctionType.Copy

    nc = bacc.Bacc("TRN2", target_bir_lowering=False, debug=False,
                   enable_asserts=False, num_devices=NCORES)

    xt_d = nc.dram_tensor("xt", [BPC, NFB, P, S], f32, kind="ExternalInput").ap()
    wt_d = nc.dram_tensor("wt", [K, NFB, P, F], f32, kind="ExternalInput").ap()
    cb_d = nc.dram_tensor("cb", [F, 1], f32, kind="ExternalInput").ap()
    yt_d = nc.dram_tensor("yt", [BPC, NFB, P, L], f32, kind="ExternalOutput").ap()

    with tile.TileContext(nc) as tc, ExitStack() as ctx:
        wpool = ctx.enter_context(tc.tile_pool(name="w", bufs=1))
        xpool = ctx.enter_context(tc.tile_pool(name="x", bufs=2))
        tpool = ctx.enter_context(tc.tile_pool(name="t", bufs=3))
        qpool = ctx.enter_context(tc.tile_pool(name="q", bufs=2))
        opool = ctx.enter_context(tc.tile_pool(name="o", bufs=2))
        ppool = ctx.enter_context(tc.tile_pool(name="p", bufs=2, space="PSUM"))

        # weights + bias: loaded once, live forever
        w_sb = []
        for k in range(K):
            row = []
            for ic in range(NFB):
                t = wpool.tile([P, F], f32r, tag=f"w{k}{ic}")
                nc.sync.dma_start(t[:], wt_d[k, ic].bitcast(f32r))
                row.append(t)
            w_sb.append(row)
        bias_sb = wpool.tile([P, NFB], f32, tag="bias")
        nc.sync.dma_start(
            bias_sb[:], cb_d.rearrange("(ob p) one -> p (ob one)", p=P))

        for b in range(BPC):
            # x^T, both feature blocks side by side: [128, 2048] fp32r
            xt = xpool.tile([P, NFB * S], f32r, tag="xt")
            for ic in range(NFB):
                nc.sync.dma_start(xt[:, ic * S:(ic + 1) * S],
                                  xt_d[b, ic].bitcast(f32r))
            # fp16 copies for the elementwise chain (xh_odd = x shifted by 1
            # so odd-shift windows stay 4B-aligned for the DVE 2x mode)
            xf = xt[:].bitcast(f32)
            xh = xpool.tile([P, NFB * S], f16, tag="xh")
            nc.scalar.activation(xh[:], xf, CPY)
            xho = xpool.tile([P, NFB * S], f16, tag="xho")
            nc.scalar.activation(xho[:, :NFB * S - 1],
                                 xt[:, 1:NFB * S].bitcast(f32), CPY)

            # conv -> tau, per output-feature block; 4 PSUM tiles per batch,
            # matmuls ordered weight-major so each LDWEIGHTS serves 2 MMs.
            pss = {}
            for ob in range(NFB):
                for li, l0 in enumerate((0, L - LT)):
                    pss[(ob, li)] = ppool.tile([P, LT], f32, tag=f"ps{ob}{li}",
                                               name=f"ps{ob}{li}_{b}")
            for ic in range(NFB):
                for k in range(K):
                    first = (ic == 0 and k == 0)
                    last = (ic == NFB - 1 and k == K - 1)
                    for ob in range(NFB):
                        for li, l0 in enumerate((0, L - LT)):
                            nc.tensor.matmul(
                                pss[(ob, li)][:],
                                w_sb[k][ic][:, ob * P:(ob + 1) * P],
                                xt[:, ic * S + l0 + k: ic * S + l0 + k + LT],
                                start=first, stop=last,
                            )

            # tau (fp16, both obs in one [128, 2048] tile at cols ob*1024)
            W2 = NFB * S
            tau = tpool.tile([P, W2], f16, tag="tau")
            for ob in range(NFB):
                for li, l0 in enumerate((0, L - LT)):
                    nc.scalar.activation(
                        tau[:, ob * S + l0: ob * S + l0 + LT],
                        pss[(ob, li)][:], SIG,
                        bias=bias_sb[:, ob:ob + 1], scale=1.0)
            t2 = tpool.tile([P, W2], f16, tag="t2")
            nc.scalar.activation(t2[:], tau[:], SQU)
            t4 = tpool.tile([P, W2], f16, tag="t4")
            nc.scalar.activation(t4[:], t2[:], SQU)

            def pair(t, off=0):
                return t[:].rearrange("p (c n) -> p c n", c=2)[:, :, off:off + LE]

            th, t2p, t4p = pair(tau), pair(t2), pair(t4)

            def weven(i):
                return pair(xh, i)

            def wodd(i):  # i odd; the shifted copy at i-1 keeps alignment
                return pair(xho, i - 1)

            # numerator chain, all fp16 2x-mode on DVE (GPSIMD unused: its
            # concurrent SBUF traffic halves DVE throughput via port sharing)
            u0 = qpool.tile([P, W2], f16, tag="u")
            nc.vector.tensor_tensor(pair(u0), th, weven(0), TT.mult)
            q0 = qpool.tile([P, W2], f16, tag="q0")
            nc.vector.tensor_tensor(pair(q0), pair(u0), wodd(1), TT.add)
            m0 = qpool.tile([P, W2], f16, tag="m")
            nc.vector.tensor_tensor(pair(m0), pair(q0), t2p, TT.mult)

            u1 = qpool.tile([P, W2], f16, tag="u")
            nc.vector.tensor_tensor(pair(u1), th, weven(2), TT.mult)
            q1 = qpool.tile([P, W2], f16, tag="q1")
            nc.vector.tensor_tensor(pair(q1), pair(u1), wodd(3), TT.add)
            h0 = qpool.tile([P, W2], f16, tag="hh")
            nc.vector.tensor_tensor(pair(h0), pair(m0), pair(q1), TT.add)
            m1 = qpool.tile([P, W2], f16, tag="m")
            nc.vector.tensor_tensor(pair(m1), pair(h0), t4p, TT.mult)

            u2 = qpool.tile([P, W2], f16, tag="u")
            nc.vector.tensor_tensor(pair(u2), th, weven(4), TT.mult)
            q2 = qpool.tile([P, W2], f16, tag="q2")
            nc.vector.tensor_tensor(pair(q2), pair(u2), wodd(5), TT.add)
            h1 = qpool.tile([P, W2], f16, tag="hh")
            nc.vector.tensor_tensor(pair(h1), pair(q2), t2p, TT.mult)

            u3 = qpool.tile([P, W2], f16, tag="u")
            nc.vector.tensor_tensor(pair(u3), th, weven(6), TT.mult)
            q3 = qpool.tile([P, W2], f16, tag="q3")
            nc.vector.tensor_tensor(pair(q3), pair(u3), wodd(7), TT.add)
            h2 = qpool.tile([P, W2], f16, tag="h2")
            nc.vector.tensor_tensor(pair(h2), pair(h1), pair(q3), TT.add)

            nh = qpool.tile([P, W2], f16, tag="nh")
            nc.vector.tensor_tensor(pair(nh), pair(m1), pair(h2), TT.add)

            # denominator + division (fp32 tail), full-width incl. junk cols
            d = opool.tile([P, W2], f32, tag="d")
            nc.vector._custom_dve(DENOM_OP, out=d[:], in0=tau[:])
            # r in fp16 (direct _custom_dve: wrapper insists on fp32 out,
            # but the NR math runs in-pipe at fp32; only the store rounds)
            from concourse.dve_ops import (RECIPROCAL_APPROX_FAST,
                                           RECIP_APPROX_FAST_CONSTS as RC)
            r = opool.tile([P, W2], f16, tag="r")
            nc.vector._custom_dve(RECIPROCAL_APPROX_FAST, out=r[:], in0=d[:],
                                  s0=RC["s0"], s1=RC["s1"], imm2=RC["imm2"])
            oh = opool.tile([P, W2], f16, tag="oh")
            nc.vector.tensor_tensor(pair(oh), pair(nh), pair(r), TT.mult)
            of = opool.tile([P, W2], f32, tag="of")
            nc.scalar.activation(of[:], oh[:], CPY)
            for ob in range(NFB):
                nc.sync.dma_start(yt_d[b, ob], of[:, ob * S: ob * S + L])

    nc.compile()
    return nc


_NC = None


def _get_nc():
    global _NC
    if _NC is None:
        _NC = build_module()
    return _NC


def prep_inputs(x, conv_w, conv_b):
    xt = np.ascontiguousarray(
        x.transpose(0, 2, 1)).astype(np.float32, copy=False)
    xt = xt.reshape(B, NFB, P, S)
    wt = np.ascontiguousarray(
        conv_w.transpose(2, 1, 0)).astype(np.float32, copy=False)
    wt = wt.reshape(K, NFB, P, F)
    cb = np.ascontiguousarray(conv_b, dtype=np.float32).reshape(F, 1)
    return xt, wt, cb


def make_in_maps(x, conv_w, conv_b):
    xt, wt, cb = prep_inputs(x, conv_w, conv_b)
    return [
        {"xt": xt[c * BPC:(c + 1) * BPC], "wt": wt, "cb": cb}
        for c in range(NCORES)
    ]


def gather_output(results):
    out = np.empty((B, L, F), np.float32)
    for c in range(NCORES):
        yt = results[c]["yt"]  # [BPC, NFB, P, L]
        out[c * BPC:(c + 1) * BPC] = (
            yt.transpose(0, 3, 1, 2).reshape(BPC, L, F))
    return out


def kernel(x, conv_w, conv_b):
    nc = _get_nc()
    in_maps = make_in_maps(x, conv_w, conv_b)
    res = run_bass_kernel_spmd(nc, in_maps, core_ids=list(range(NCORES)))
    return gather_output(res.results)



# revision 7
# speedup vs baseline: 1.0993x; 1.0993x over previous
"""Trainium2 Bass kernel for nn_ConvFilter (geometric-series conv filter).

Math (per batch b, output position l, feature f):
    t[o,l]  = sum_{i,k} conv_w[o,i,k] * x[l+k,i]          (valid conv, L=S-K+1)
    tau     = sigmoid(t + bias)
    out     = (sum_i tau^(7-i) * x[l+i,f]) / (sum_i tau^i)

Implementation (v2 — fp16 end-to-end on device):
  * host pre-transposes x to [feature, seq] fp16 (aligned + 1-shifted copy
    so every even/odd window stays 4-byte aligned for the DVE 2x mode);
    weights converted to fp16 on host. The SAME fp16 x tiles feed both the
    PE conv (fp16 matmul, full rate) and the DVE elementwise chain.
  * conv: 16 accumulating fp16 matmuls per 512-wide l-tile; two overlapping
    l-tiles (0 and L-512) per output-feature block; PSUM fp32.
  * tau = sigmoid(psum+bias) on ACT (fp16 out); T2 = tau^2, T4 = T2^2 on ACT.
  * numerator via Estrin in fp16 2x mode on DVE:
        q_j = tau*x_{2j} + x_{2j+1}
        N   = (q0*T2 + q1)*T4 + (q2*T2 + q3)
  * 1/denominator as ONE custom DVE op: r = p5(tau) where p5 is the degree-5
    relative-minimax fit of 1/((1+t)(1+t^2)(1+t^4)) on [0,1] with p(0)=1,
    p'(0)=-1 pinned (max rel err 1.4e-3). out = N * r in fp16.
  * output DMA'd as fp16; host converts to fp32.
  * data-parallel over batch: 8 batches/core on 8 cores, weights replicated.
"""

import numpy as np
from contextlib import ExitStack

import concourse.bass as bass
import concourse.tile as tile
from concourse import bacc, mybir
from concourse.bass_utils import run_bass_kernel_spmd
from concourse import dve_ops
from concourse.dve_ops import DveOp
from concourse.dve_spec import (
    Spec, Src0, C0, C1, C2, C3, One, sq, lower, _has_src1, _spill_c3_to_src1,
)
from concourse.dve_uop import DveOpSpec

B, S, F, K = 64, 1024, 256, 8
L = S - K + 1  # 1017
NCORES = 8
BPC = B // NCORES
P = 128
NFB = F // P  # 2 feature blocks
LT = 512      # matmul l-tile width (one PSUM bank)
LE = L + 1    # even fp16 elementwise width (DVE 2x mode needs even counts)
W2 = NFB * S  # 2048: both feature blocks side by side

# degree-4 relative-minimax fit of 1/((1+t)(1+t^2)(1+t^4)) on [0,1] with
# p(0)=1 pinned; max rel err 2.66e-3.
RP_C1 = -0.9619214
RP_C2 = -0.1795749
RP_C3 = 0.156929
RP_C4 = 0.1098995


def _register_op(name, spec, subdim=False):
    for existing in dve_ops.OPS:
        if existing.name == name:
            return existing
    shas = {}
    for ver in ("v3", "v4"):
        tmp = DveOpSpec(name=name, opcode=0, uops=lower(spec, ver=ver),
                        rd1_en=_has_src1(spec))
        shas[ver] = tmp.sha(ver)
    op = DveOp(name, spec, subdim=subdim, uops_sha=shas)
    dve_ops.OPS.append(op)
    dve_ops.CUSTOM_DVE_SPECS[name] = spec
    dve_ops._SUB_OPCODE_FOR_NAME[name] = (
        dve_ops._CUSTOM_DVE_ROW_BASE + len(dve_ops.OPS) - 1
    )
    assert dve_ops._SUB_OPCODE_FOR_NAME[name] < 0x20
    return op


def _get_rpoly_op():
    # r = 1 + t(c1 + t(c2 + t(c3 + c4 t)))  — Horner, c4 spilled to Src1.
    t = Src0
    h = C3 * t
    h = C2 + h
    h = h * t
    h = C1 + h
    h = h * t
    h = C0 + h
    h = h * t
    body = One + h

    def _ref(in0, in1, s0, s1, imm2):
        c4 = in1.reshape(in1.shape[0], -1)[:, :1]
        tt = in0.astype(np.float32)
        return (1.0 + tt * (s0 + tt * (s1 + tt * (imm2 + c4 * tt)))
                ).astype(np.float32)

    spec = Spec(body=_spill_c3_to_src1(body), reference=_ref)
    return _register_op("ANT_CF_RPOLY4", spec)


def build_module():
    RPOLY = _get_rpoly_op()
    f32 = mybir.dt.float32
    f16 = mybir.dt.float16
    TT = mybir.AluOpType
    SIG = mybir.ActivationFunctionType.Sigmoid
    SQU = mybir.ActivationFunctionType.Square

    nc = bacc.Bacc("TRN2", target_bir_lowering=False, debug=False,
                   enable_asserts=False, num_devices=NCORES)

    xh_d = nc.dram_tensor("xh", [BPC, P, W2], f16, kind="ExternalInput").ap()
    xo_d = nc.dram_tensor("xo", [BPC, P, W2], f16, kind="ExternalInput").ap()
    wt_d = nc.dram_tensor("wt", [K, NFB, P, F], f16, kind="ExternalInput").ap()
    cb_d = nc.dram_tensor("cb", [F, 1], f32, kind="ExternalInput").ap()
    yt_d = nc.dram_tensor("yt", [BPC, NFB, P, L], f16, kind="ExternalOutput").ap()

    with tile.TileContext(nc) as tc, ExitStack() as ctx:
        wpool = ctx.enter_context(tc.tile_pool(name="w", bufs=1))
        xpool = ctx.enter_context(tc.tile_pool(name="x", bufs=2))
        tpool = ctx.enter_context(tc.tile_pool(name="t", bufs=2))
        qpool = ctx.enter_context(tc.tile_pool(name="q", bufs=2))
        opool = ctx.enter_context(tc.tile_pool(name="o", bufs=2))
        ppool = ctx.enter_context(tc.tile_pool(name="p", bufs=2, space="PSUM"))

        # weights + bias + rpoly c5 constant: loaded once, live forever
        w_sb = []
        for k in range(K):
            row = []
            for ic in range(NFB):
                t = wpool.tile([P, F], f16, tag=f"w{k}{ic}")
                nc.sync.dma_start(t[:], wt_d[k, ic])
                row.append(t)
            w_sb.append(row)
        bias_sb = wpool.tile([P, NFB], f32, tag="bias")
        nc.sync.dma_start(
            bias_sb[:], cb_d.rearrange("(ob p) one -> p (ob one)", p=P))
        c4_sb = wpool.tile([P, 1], f32, tag="c4")
        nc.gpsimd.memset(c4_sb[:], RP_C4)

        def pair(t, off=0):
            return t[:].rearrange("p (c n) -> p c n", c=2)[:, :, off:off + LE]

        for b in range(BPC):
            # x^T fp16, both feature blocks side by side: [128, 2048]
            # xh: aligned copy; xo: 1-left-shifted copy (odd windows)
            xh = xpool.tile([P, W2], f16, tag="xh")
            nc.sync.dma_start(xh[:], xh_d[b])
            xo = xpool.tile([P, W2], f16, tag="xo")
            nc.sync.dma_start(xo[:], xo_d[b])

            # conv -> 4 PSUM tiles per batch (2 out-blocks x 2 l-tiles)
            pss = {}
            for ob in range(NFB):
                for li, l0 in enumerate((0, L - LT)):
                    pss[(ob, li)] = ppool.tile([P, LT], f32, tag=f"ps{ob}{li}",
                                               name=f"ps{ob}{li}_{b}")
            for ic in range(NFB):
                for k in range(K):
                    first = (ic == 0 and k == 0)
                    last = (ic == NFB - 1 and k == K - 1)
                    for ob in range(NFB):
                        for li, l0 in enumerate((0, L - LT)):
                            nc.tensor.matmul(
                                pss[(ob, li)][:],
                                w_sb[k][ic][:, ob * P:(ob + 1) * P],
                                xh[:, ic * S + l0 + k: ic * S + l0 + k + LT],
                                start=first, stop=last,
                            )

            # tau (fp16, both obs in one [128, 2048] tile at cols ob*1024)
            tau = tpool.tile([P, W2], f16, tag="tau")
            for ob in range(NFB):
                for li, l0 in enumerate((0, L - LT)):
                    nc.scalar.activation(
                        tau[:, ob * S + l0: ob * S + l0 + LT],
                        pss[(ob, li)][:], SIG,
                        bias=bias_sb[:, ob:ob + 1], scale=1.0)

            # r = 1/denominator via the degree-5 poly custom op (DVE, 1x).
            # Issued first: it only needs tau, and runs while ACT does T2/T4.
            r = opool.tile([P, W2], f16, tag="r")
            nc.vector._custom_dve(RPOLY, out=r[:], in0=tau[:], in1=c4_sb[:],
                                  s0=RP_C1, s1=RP_C2, imm2=RP_C3)

            t2 = tpool.tile([P, W2], f16, tag="t2")
            nc.scalar.activation(t2[:], tau[:], SQU)
            t4 = tpool.tile([P, W2], f16, tag="t4")
            nc.scalar.activation(t4[:], t2[:], SQU)

            th, t2p, t4p = pair(tau), pair(t2), pair(t4)

            def weven(i):
                return pair(xh, i)

            def wodd(i):  # i odd; the shifted copy at i-1 keeps alignment
                return pair(xo, i - 1)

            # numerator chain, all fp16 2x-mode on DVE
            u0 = qpool.tile([P, W2], f16, tag="u")
            nc.vector.tensor_tensor(pair(u0), th, weven(0), TT.mult)
            q0 = qpool.tile([P, W2], f16, tag="q0")
            nc.vector.tensor_tensor(pair(q0), pair(u0), wodd(1), TT.add)
            m0 = qpool.tile([P, W2], f16, tag="m")
            nc.vector.tensor_tensor(pair(m0), pair(q0), t2p, TT.mult)

            u1 = qpool.tile([P, W2], f16, tag="u")
            nc.vector.tensor_tensor(pair(u1), th, weven(2), TT.mult)
            q1 = qpool.tile([P, W2], f16, tag="q1")
            nc.vector.tensor_tensor(pair(q1), pair(u1), wodd(3), TT.add)
            h0 = qpool.tile([P, W2], f16, tag="hh")
            nc.vector.tensor_tensor(pair(h0), pair(m0), pair(q1), TT.add)
            m1 = qpool.tile([P, W2], f16, tag="m")
            nc.vector.tensor_tensor(pair(m1), pair(h0), t4p, TT.mult)

            u2 = qpool.tile([P, W2], f16, tag="u")
            nc.vector.tensor_tensor(pair(u2), th, weven(4), TT.mult)
            q2 = qpool.tile([P, W2], f16, tag="q2")
            nc.vector.tensor_tensor(pair(q2), pair(u2), wodd(5), TT.add)
            h1 = qpool.tile([P, W2], f16, tag="hh")
            nc.vector.tensor_tensor(pair(h1), pair(q2), t2p, TT.mult)

            u3 = qpool.tile([P, W2], f16, tag="u")
            nc.vector.tensor_tensor(pair(u3), th, weven(6), TT.mult)
            q3 = qpool.tile([P, W2], f16, tag="q3")
            nc.vector.tensor_tensor(pair(q3), pair(u3), wodd(7), TT.add)
            h2 = qpool.tile([P, W2], f16, tag="h2")
            nc.vector.tensor_tensor(pair(h2), pair(h1), pair(q3), TT.add)

            nh = qpool.tile([P, W2], f16, tag="nh")
            nc.vector.tensor_tensor(pair(nh), pair(m1), pair(h2), TT.add)

            # out = N * r (fp16), DMA'd as fp16; host converts to fp32
            oh = opool.tile([P, W2], f16, tag="oh")
            nc.vector.tensor_tensor(pair(oh), pair(nh), pair(r), TT.mult)
            for ob in range(NFB):
                nc.sync.dma_start(yt_d[b, ob], oh[:, ob * S: ob * S + L])

    nc.compile()
    return nc


_NC = None


def _get_nc():
    global _NC
    if _NC is None:
        _NC = build_module()
    return _NC


def prep_inputs(x, conv_w, conv_b):
    xt = np.ascontiguousarray(
        np.asarray(x).transpose(0, 2, 1)).astype(np.float16)
    xh = xt.reshape(B, NFB * P, S).reshape(B, NFB, P, S)
    # interleave the two feature blocks side by side: [B, P, NFB*S]
    xh = np.ascontiguousarray(xh.transpose(0, 2, 1, 3)).reshape(B, P, W2)
    xo = np.empty_like(xh)
    xo[:, :, :W2 - 1] = xh[:, :, 1:]
    xo[:, :, W2 - 1] = 0
    wt = np.ascontiguousarray(
        np.asarray(conv_w).transpose(2, 1, 0)).astype(np.float16)
    wt = wt.reshape(K, NFB, P, F)
    cb = np.ascontiguousarray(conv_b, dtype=np.float32).reshape(F, 1)
    return xh, xo, wt, cb


def make_in_maps(x, conv_w, conv_b):
    xh, xo, wt, cb = prep_inputs(x, conv_w, conv_b)
    return [
        {"xh": xh[c * BPC:(c + 1) * BPC], "xo": xo[c * BPC:(c + 1) * BPC],
         "wt": wt, "cb": cb}
        for c in range(NCORES)
    ]


def gather_output(results):
    out = np.empty((B, L, F), np.float32)
    for c in range(NCORES):
        yt = results[c]["yt"].astype(np.float32)  # [BPC, NFB, P, L]
        out[c * BPC:(c + 1) * BPC] = (
            yt.transpose(0, 3, 1, 2).reshape(BPC, L, F))
    return out


def kernel(x, conv_w, conv_b):
    nc = _get_nc()
    in_maps = make_in_maps(x, conv_w, conv_b)
    res = run_bass_kernel_spmd(nc, in_maps, core_ids=list(range(NCORES)))
    return gather_output(res.results)


# revision 9
# speedup vs baseline: 1.1828x; 1.0760x over previous
"""Trainium2 Bass kernel for nn_ConvFilter (geometric-series conv filter).

Math (per batch b, output position l, feature f):
    t[o,l]  = sum_{i,k} conv_w[o,i,k] * x[l+k,i]          (valid conv, L=S-K+1)
    tau     = sigmoid(t + bias)
    out     = (sum_i tau^(7-i) * x[l+i,f]) / (sum_i tau^i)

Implementation (v2 — fp16 end-to-end on device):
  * host pre-transposes x to [feature, seq] fp16 (aligned + 1-shifted copy
    so every even/odd window stays 4-byte aligned for the DVE 2x mode);
    weights converted to fp16 on host. The SAME fp16 x tiles feed both the
    PE conv (fp16 matmul, full rate) and the DVE elementwise chain.
  * conv: 16 accumulating fp16 matmuls per 512-wide l-tile; two overlapping
    l-tiles (0 and L-512) per output-feature block; PSUM fp32.
  * tau = sigmoid(psum+bias) on ACT (fp16 out); T2 = tau^2, T4 = T2^2 on ACT.
  * numerator via Estrin in fp16 2x mode on DVE:
        q_j = tau*x_{2j} + x_{2j+1}
        N   = (q0*T2 + q1)*T4 + (q2*T2 + q3)
  * 1/denominator as ONE custom DVE op: r = p5(tau) where p5 is the degree-5
    relative-minimax fit of 1/((1+t)(1+t^2)(1+t^4)) on [0,1] with p(0)=1,
    p'(0)=-1 pinned (max rel err 1.4e-3). out = N * r in fp16.
  * output DMA'd as fp16; host converts to fp32.
  * data-parallel over batch: 8 batches/core on 8 cores, weights replicated.
"""

import numpy as np
from contextlib import ExitStack

import concourse.bass as bass
import concourse.tile as tile
from concourse import bacc, mybir
from concourse.bass_utils import run_bass_kernel_spmd
from concourse import dve_ops
from concourse.dve_ops import DveOp
from concourse.dve_spec import (
    Spec, Src0, C0, C1, C2, C3, One, sq, lower, _has_src1, _spill_c3_to_src1,
)
from concourse.dve_uop import DveOpSpec

B, S, F, K = 64, 1024, 256, 8
L = S - K + 1  # 1017
NCORES = 8
BPC = B // NCORES
P = 128
NFB = F // P  # 2 feature blocks
LT = 512      # matmul l-tile width (one PSUM bank)
LE = L + 1    # even fp16 elementwise width (DVE 2x mode needs even counts)
W2 = NFB * S  # 2048: both feature blocks side by side

# degree-4 relative-minimax fit of 1/((1+t)(1+t^2)(1+t^4)) on [0,1] with
# p(0)=1 pinned; max rel err 2.66e-3.
RP_C1 = -0.9619214
RP_C2 = -0.1795749
RP_C3 = 0.156929
RP_C4 = 0.1098995


def _register_op(name, spec, subdim=False):
    for existing in dve_ops.OPS:
        if existing.name == name:
            return existing
    shas = {}
    for ver in ("v3", "v4"):
        tmp = DveOpSpec(name=name, opcode=0, uops=lower(spec, ver=ver),
                        rd1_en=_has_src1(spec))
        shas[ver] = tmp.sha(ver)
    op = DveOp(name, spec, subdim=subdim, uops_sha=shas)
    dve_ops.OPS.append(op)
    dve_ops.CUSTOM_DVE_SPECS[name] = spec
    dve_ops._SUB_OPCODE_FOR_NAME[name] = (
        dve_ops._CUSTOM_DVE_ROW_BASE + len(dve_ops.OPS) - 1
    )
    assert dve_ops._SUB_OPCODE_FOR_NAME[name] < 0x20
    return op


def _get_rpoly_op():
    # r = 1 + t(c1 + t(c2 + t(c3 + c4 t)))  — Horner, c4 spilled to Src1.
    t = Src0
    h = C3 * t
    h = C2 + h
    h = h * t
    h = C1 + h
    h = h * t
    h = C0 + h
    h = h * t
    body = One + h

    def _ref(in0, in1, s0, s1, imm2):
        c4 = in1.reshape(in1.shape[0], -1)[:, :1]
        tt = in0.astype(np.float32)
        return (1.0 + tt * (s0 + tt * (s1 + tt * (imm2 + c4 * tt)))
                ).astype(np.float32)

    spec = Spec(body=_spill_c3_to_src1(body), reference=_ref)
    return _register_op("ANT_CF_RPOLY4", spec)


def build_module():
    RPOLY = _get_rpoly_op()
    f32 = mybir.dt.float32
    f16 = mybir.dt.float16
    TT = mybir.AluOpType
    SIG = mybir.ActivationFunctionType.Sigmoid
    SQU = mybir.ActivationFunctionType.Square

    nc = bacc.Bacc("TRN2", target_bir_lowering=False, debug=False,
                   enable_asserts=False, num_devices=NCORES)

    xh_d = nc.dram_tensor("xh", [BPC, P, W2], f16, kind="ExternalInput").ap()
    xo_d = nc.dram_tensor("xo", [BPC, P, W2], f16, kind="ExternalInput").ap()
    wt_d = nc.dram_tensor("wt", [K, NFB, P, F], f16, kind="ExternalInput").ap()
    cb_d = nc.dram_tensor("cb", [F, 1], f32, kind="ExternalInput").ap()
    yt_d = nc.dram_tensor("yt", [BPC, NFB, P, L], f16, kind="ExternalOutput").ap()

    with tile.TileContext(nc) as tc, ExitStack() as ctx:
        wpool = ctx.enter_context(tc.tile_pool(name="w", bufs=1))
        xpool = ctx.enter_context(tc.tile_pool(name="x", bufs=2))
        tpool = ctx.enter_context(tc.tile_pool(name="t", bufs=2))
        qpool = ctx.enter_context(tc.tile_pool(name="q", bufs=2))
        opool = ctx.enter_context(tc.tile_pool(name="o", bufs=2))
        ppool = ctx.enter_context(tc.tile_pool(name="p", bufs=2, space="PSUM"))

        def load_x(b):
            # x^T fp16, both feature blocks side by side: [128, 2048]
            # xh: aligned copy; xo: 1-left-shifted copy (odd windows).
            # xh split per feature block so the ic=0 matmuls start as soon
            # as the first half lands.
            xh = xpool.tile([P, W2], f16, tag="xh")
            nc.sync.dma_start(xh[:, :S], xh_d[b][:, :S])
            nc.sync.dma_start(xh[:, S:], xh_d[b][:, S:])
            xo = xpool.tile([P, W2], f16, tag="xo")
            nc.sync.dma_start(xo[:], xo_d[b])
            return xh, xo

        # batch-0 x first (the first matmuls block on it), then weights in
        # matmul consumption order (ic-major), then the rest.
        x0 = load_x(0)
        w_sb = [[None] * NFB for _ in range(K)]
        for ic in range(NFB):
            for k in range(K):
                t = wpool.tile([P, F], f16, tag=f"w{k}{ic}")
                nc.sync.dma_start(t[:], wt_d[k, ic])
                w_sb[k][ic] = t
        bias_sb = wpool.tile([P, NFB], f32, tag="bias")
        nc.sync.dma_start(
            bias_sb[:], cb_d.rearrange("(ob p) one -> p (ob one)", p=P))
        c4_sb = wpool.tile([P, 1], f32, tag="c4")
        nc.gpsimd.memset(c4_sb[:], RP_C4)

        def pair(t, off=0):
            return t[:].rearrange("p (c n) -> p c n", c=2)[:, :, off:off + LE]

        def winview(t):
            # overlapping even windows of a [P, W2] tile:
            # [P, j:4 (stride 2), c:2 (stride S), n:LE (stride 1)]
            ap = t[:].rearrange("p (j c n) -> p j c n", j=4, c=2)
            raw = ap.ap
            raw[1] = [2, 4]
            raw[2] = [S, 2]
            raw[3] = [1, LE]
            ap.ap = raw
            return ap

        def repview(t, nj):
            # [P, W2] tile repeated nj times along a stride-0 j dim
            ap = t[:].rearrange("p (j c n) -> p j c n", j=nj, c=2)
            raw = ap.ap
            raw[1] = [0, nj]
            raw[2] = [S, 2]
            raw[3] = [1, LE]
            ap.ap = raw
            return ap

        def view4(t):   # natural [P, 4, 2, LE] view of a [P, 4*W2] tile
            return t[:].rearrange("p (j c n) -> p j c n", j=4, c=2)[:, :, :, :LE]

        def qsel(t, start):  # j in {start, start+2} of a [P, 4*W2] tile
            return t[:].rearrange(
                "p (a j c n) -> p a j c n", a=2, j=2, c=2
            )[:, :, start, :, :LE]

        def view2(t):   # natural [P, 2, 2, LE] view of a [P, 2*W2] tile
            return t[:].rearrange("p (j c n) -> p j c n", j=2, c=2)[:, :, :, :LE]

        def jslice(t, j):  # single j of a [P, 2*W2] tile -> [P, 2, LE]
            return t[:].rearrange("p (j c n) -> p j c n", j=2, c=2)[:, j, :, :LE]

        for b in range(BPC):
            xh, xo = x0 if b == 0 else load_x(b)

            # conv -> 4 PSUM tiles per batch (2 out-blocks x 2 l-tiles)
            pss = {}
            for ob in range(NFB):
                for li, l0 in enumerate((0, L - LT)):
                    pss[(ob, li)] = ppool.tile([P, LT], f32, tag=f"ps{ob}{li}",
                                               name=f"ps{ob}{li}_{b}")
            for ic in range(NFB):
                for k in range(K):
                    first = (ic == 0 and k == 0)
                    last = (ic == NFB - 1 and k == K - 1)
                    for ob in range(NFB):
                        for li, l0 in enumerate((0, L - LT)):
                            nc.tensor.matmul(
                                pss[(ob, li)][:],
                                w_sb[k][ic][:, ob * P:(ob + 1) * P],
                                xh[:, ic * S + l0 + k: ic * S + l0 + k + LT],
                                start=first, stop=last,
                            )

            # tau (fp16, both obs in one [128, 2048] tile at cols ob*1024)
            tau = tpool.tile([P, W2], f16, tag="tau")
            for ob in range(NFB):
                for li, l0 in enumerate((0, L - LT)):
                    nc.scalar.activation(
                        tau[:, ob * S + l0: ob * S + l0 + LT],
                        pss[(ob, li)][:], SIG,
                        bias=bias_sb[:, ob:ob + 1], scale=1.0)

            # r = 1/denominator via the degree-5 poly custom op (DVE, 1x).
            # Issued first: it only needs tau, and runs while ACT does T2/T4.
            r = opool.tile([P, W2], f16, tag="r")
            nc.vector._custom_dve(RPOLY, out=r[:], in0=tau[:], in1=c4_sb[:],
                                  s0=RP_C1, s1=RP_C2, imm2=RP_C3)

            t2 = tpool.tile([P, W2], f16, tag="t2")
            nc.scalar.activation(t2[:], tau[:], SQU)
            t4 = tpool.tile([P, W2], f16, tag="t4")
            nc.scalar.activation(t4[:], t2[:], SQU)

            # numerator chain, fp16 2x-mode on DVE, window-batched:
            #   u_all[j] = tau * x_{2j}           (one op, 4 windows)
            #   q_all[j] = u_all[j] + x_{2j+1}    (one op, 4 windows)
            #   mh = [q0, q2] * T2  -> [m0, h1]   (one op, 2 lanes)
            #   hh = mh + [q1, q3]  -> [h0, h2]   (one op, 2 lanes)
            #   m1 = h0 * T4;  N = m1 + h2
            u_all = qpool.tile([P, 4 * W2], f16, tag="u4")
            nc.vector.tensor_tensor(view4(u_all), repview(tau, 4),
                                    winview(xh), TT.mult)
            q_all = qpool.tile([P, 4 * W2], f16, tag="q4")
            nc.vector.tensor_tensor(view4(q_all), view4(u_all),
                                    winview(xo), TT.add)
            mh = qpool.tile([P, 2 * W2], f16, tag="mh")
            nc.vector.tensor_tensor(view2(mh), qsel(q_all, 0),
                                    repview(t2, 2), TT.mult)
            hh = qpool.tile([P, 2 * W2], f16, tag="hh")
            nc.vector.tensor_tensor(view2(hh), view2(mh),
                                    qsel(q_all, 1), TT.add)
            m1 = qpool.tile([P, W2], f16, tag="m1")
            nc.vector.tensor_tensor(pair(m1), jslice(hh, 0), pair(t4), TT.mult)
            nh = qpool.tile([P, W2], f16, tag="nh")
            nc.vector.tensor_tensor(pair(nh), pair(m1), jslice(hh, 1), TT.add)

            # out = N * r (fp16), DMA'd as fp16; host converts to fp32
            oh = opool.tile([P, W2], f16, tag="oh")
            nc.vector.tensor_tensor(pair(oh), pair(nh), pair(r), TT.mult)
            for ob in range(NFB):
                nc.sync.dma_start(yt_d[b, ob], oh[:, ob * S: ob * S + L])

    nc.compile()
    return nc


_NC = None


def _get_nc():
    global _NC
    if _NC is None:
        _NC = build_module()
    return _NC


def prep_inputs(x, conv_w, conv_b):
    xt = np.ascontiguousarray(
        np.asarray(x).transpose(0, 2, 1)).astype(np.float16)
    xh = xt.reshape(B, NFB * P, S).reshape(B, NFB, P, S)
    # interleave the two feature blocks side by side: [B, P, NFB*S]
    xh = np.ascontiguousarray(xh.transpose(0, 2, 1, 3)).reshape(B, P, W2)
    xo = np.empty_like(xh)
    xo[:, :, :W2 - 1] = xh[:, :, 1:]
    xo[:, :, W2 - 1] = 0
    wt = np.ascontiguousarray(
        np.asarray(conv_w).transpose(2, 1, 0)).astype(np.float16)
    wt = wt.reshape(K, NFB, P, F)
    cb = np.ascontiguousarray(conv_b, dtype=np.float32).reshape(F, 1)
    return xh, xo, wt, cb


def make_in_maps(x, conv_w, conv_b):
    xh, xo, wt, cb = prep_inputs(x, conv_w, conv_b)
    return [
        {"xh": xh[c * BPC:(c + 1) * BPC], "xo": xo[c * BPC:(c + 1) * BPC],
         "wt": wt, "cb": cb}
        for c in range(NCORES)
    ]


def gather_output(results):
    out = np.empty((B, L, F), np.float32)
    for c in range(NCORES):
        yt = results[c]["yt"].astype(np.float32)  # [BPC, NFB, P, L]
        out[c * BPC:(c + 1) * BPC] = (
            yt.transpose(0, 3, 1, 2).reshape(BPC, L, F))
    return out


def kernel(x, conv_w, conv_b):
    nc = _get_nc()
    in_maps = make_in_maps(x, conv_w, conv_b)
    res = run_bass_kernel_spmd(nc, in_maps, core_ids=list(range(NCORES)))
    return gather_output(res.results)


# revision 11
# speedup vs baseline: 1.2154x; 1.0275x over previous
"""Trainium2 Bass kernel for nn_ConvFilter (geometric-series conv filter).

Math (per batch b, output position l, feature f):
    t[o,l]  = sum_{i,k} conv_w[o,i,k] * x[l+k,i]          (valid conv, L=S-K+1)
    tau     = sigmoid(t + bias)
    out     = (sum_i tau^(7-i) * x[l+i,f]) / (sum_i tau^i)

Implementation (v2 — fp16 end-to-end on device):
  * host pre-transposes x to [feature, seq] fp16 (aligned + 1-shifted copy
    so every even/odd window stays 4-byte aligned for the DVE 2x mode);
    weights converted to fp16 on host. The SAME fp16 x tiles feed both the
    PE conv (fp16 matmul, full rate) and the DVE elementwise chain.
  * conv: 16 accumulating fp16 matmuls per 512-wide l-tile; two overlapping
    l-tiles (0 and L-512) per output-feature block; PSUM fp32.
  * tau = sigmoid(psum+bias) on ACT (fp16 out); T2 = tau^2, T4 = T2^2 on ACT.
  * numerator via Estrin in fp16 2x mode on DVE:
        q_j = tau*x_{2j} + x_{2j+1}
        N   = (q0*T2 + q1)*T4 + (q2*T2 + q3)
  * 1/denominator as ONE custom DVE op: r = p5(tau) where p5 is the degree-5
    relative-minimax fit of 1/((1+t)(1+t^2)(1+t^4)) on [0,1] with p(0)=1,
    p'(0)=-1 pinned (max rel err 1.4e-3). out = N * r in fp16.
  * output DMA'd as fp16; host converts to fp32.
  * data-parallel over batch: 8 batches/core on 8 cores, weights replicated.
"""

import numpy as np
from contextlib import ExitStack

import concourse.bass as bass
import concourse.tile as tile
from concourse import bacc, mybir
from concourse.bass_utils import run_bass_kernel_spmd
from concourse import dve_ops
from concourse.dve_ops import DveOp
from concourse.dve_spec import (
    Spec, Src0, C0, C1, C2, C3, One, sq, lower, _has_src1, _spill_c3_to_src1,
)
from concourse.dve_uop import DveOpSpec

B, S, F, K = 64, 1024, 256, 8
L = S - K + 1  # 1017
NCORES = 8
BPC = B // NCORES
P = 128
NFB = F // P  # 2 feature blocks
LT = 512      # matmul l-tile width (one PSUM bank)
LE = L + 1    # even fp16 elementwise width (DVE 2x mode needs even counts)
W2 = NFB * S  # 2048: both feature blocks side by side

# degree-4 relative-minimax fit of 1/((1+t)(1+t^2)(1+t^4)) on [0,1] with
# p(0)=1 pinned; max rel err 2.66e-3.
RP_C1 = -0.9619214
RP_C2 = -0.1795749
RP_C3 = 0.156929
RP_C4 = 0.1098995


def _register_op(name, spec, subdim=False):
    for existing in dve_ops.OPS:
        if existing.name == name:
            return existing
    shas = {}
    for ver in ("v3", "v4"):
        tmp = DveOpSpec(name=name, opcode=0, uops=lower(spec, ver=ver),
                        rd1_en=_has_src1(spec))
        shas[ver] = tmp.sha(ver)
    op = DveOp(name, spec, subdim=subdim, uops_sha=shas)
    dve_ops.OPS.append(op)
    dve_ops.CUSTOM_DVE_SPECS[name] = spec
    dve_ops._SUB_OPCODE_FOR_NAME[name] = (
        dve_ops._CUSTOM_DVE_ROW_BASE + len(dve_ops.OPS) - 1
    )
    assert dve_ops._SUB_OPCODE_FOR_NAME[name] < 0x20
    return op


def _get_rpoly_op():
    # r = 1 + t(c1 + t(c2 + t(c3 + c4 t)))  — Horner, c4 spilled to Src1.
    t = Src0
    h = C3 * t
    h = C2 + h
    h = h * t
    h = C1 + h
    h = h * t
    h = C0 + h
    h = h * t
    body = One + h

    def _ref(in0, in1, s0, s1, imm2):
        c4 = in1.reshape(in1.shape[0], -1)[:, :1]
        tt = in0.astype(np.float32)
        return (1.0 + tt * (s0 + tt * (s1 + tt * (imm2 + c4 * tt)))
                ).astype(np.float32)

    spec = Spec(body=_spill_c3_to_src1(body), reference=_ref)
    return _register_op("ANT_CF_RPOLY4", spec)


def build_module():
    RPOLY = _get_rpoly_op()
    f32 = mybir.dt.float32
    f16 = mybir.dt.float16
    TT = mybir.AluOpType
    SIG = mybir.ActivationFunctionType.Sigmoid
    SQU = mybir.ActivationFunctionType.Square

    nc = bacc.Bacc("TRN2", target_bir_lowering=False, debug=False,
                   enable_asserts=False, num_devices=NCORES)

    xh_d = nc.dram_tensor("xh", [BPC, P, W2], f16, kind="ExternalInput").ap()
    xo_d = nc.dram_tensor("xo", [BPC, P, W2], f16, kind="ExternalInput").ap()
    wt_d = nc.dram_tensor("wt", [K, NFB, P, F], f16, kind="ExternalInput").ap()
    cb_d = nc.dram_tensor("cb", [F, 1], f32, kind="ExternalInput").ap()
    yt_d = nc.dram_tensor("yt", [BPC, NFB, P, L], f16, kind="ExternalOutput").ap()

    with tile.TileContext(nc) as tc, ExitStack() as ctx:
        wpool = ctx.enter_context(tc.tile_pool(name="w", bufs=1))
        xpool = ctx.enter_context(tc.tile_pool(name="x", bufs=2))
        tpool = ctx.enter_context(tc.tile_pool(name="t", bufs=2))
        qpool = ctx.enter_context(tc.tile_pool(name="q", bufs=2))
        opool = ctx.enter_context(tc.tile_pool(name="o", bufs=2))
        ppool = ctx.enter_context(tc.tile_pool(name="p", bufs=2, space="PSUM"))

        def load_x(b):
            # x^T fp16, both feature blocks side by side: [128, 2048]
            # xh: aligned copy; xo: 1-left-shifted copy (odd windows).
            # xh split per feature block so the ic=0 matmuls start as soon
            # as the first half lands.
            xh = xpool.tile([P, W2], f16, tag="xh")
            nc.sync.dma_start(xh[:, :S], xh_d[b][:, :S])
            nc.sync.dma_start(xh[:, S:], xh_d[b][:, S:])
            xo = xpool.tile([P, W2], f16, tag="xo")
            nc.sync.dma_start(xo[:], xo_d[b])
            return xh, xo

        # batch-0 x first (the first matmuls block on it), then weights in
        # matmul consumption order (ic-major), then the rest.
        x0 = load_x(0)
        w_sb = [[None] * NFB for _ in range(K)]
        for ic in range(NFB):
            for k in range(K):
                t = wpool.tile([P, F], f16, tag=f"w{k}{ic}")
                nc.sync.dma_start(t[:], wt_d[k, ic])
                w_sb[k][ic] = t
        bias_sb = wpool.tile([P, NFB], f32, tag="bias")
        nc.sync.dma_start(
            bias_sb[:], cb_d.rearrange("(ob p) one -> p (ob one)", p=P))
        c4_sb = wpool.tile([P, 1], f32, tag="c4")
        nc.gpsimd.memset(c4_sb[:], RP_C4)

        def pair(t, off=0):
            return t[:].rearrange("p (c n) -> p c n", c=2)[:, :, off:off + LE]

        def winview(t):
            # overlapping even windows of a [P, W2] tile:
            # [P, j:4 (stride 2), c:2 (stride S), n:LE (stride 1)]
            ap = t[:].rearrange("p (j c n) -> p j c n", j=4, c=2)
            raw = ap.ap
            raw[1] = [2, 4]
            raw[2] = [S, 2]
            raw[3] = [1, LE]
            ap.ap = raw
            return ap

        def repview(t, nj):
            # [P, W2] tile repeated nj times along a stride-0 j dim
            ap = t[:].rearrange("p (j c n) -> p j c n", j=nj, c=2)
            raw = ap.ap
            raw[1] = [0, nj]
            raw[2] = [S, 2]
            raw[3] = [1, LE]
            ap.ap = raw
            return ap

        def view4(t):   # natural [P, 4, 2, LE] view of a [P, 4*W2] tile
            return t[:].rearrange("p (j c n) -> p j c n", j=4, c=2)[:, :, :, :LE]

        def qsel(t, start):  # j in {start, start+2} of a [P, 4*W2] tile
            return t[:].rearrange(
                "p (a j c n) -> p a j c n", a=2, j=2, c=2
            )[:, :, start, :, :LE]

        def view2(t):   # natural [P, 2, 2, LE] view of a [P, 2*W2] tile
            return t[:].rearrange("p (j c n) -> p j c n", j=2, c=2)[:, :, :, :LE]

        def jslice(t, j):  # single j of a [P, 2*W2] tile -> [P, 2, LE]
            return t[:].rearrange("p (j c n) -> p j c n", j=2, c=2)[:, j, :, :LE]

        for b in range(BPC):
            xh, xo = x0 if b == 0 else load_x(b)

            # conv -> 4 PSUM tiles per batch (2 out-blocks x 2 l-tiles).
            # Batch 0 runs group-major so fb0's tau is ready ~10us earlier;
            # steady state runs weight-major (each LDWEIGHTS feeds 4 MMs).
            pss = {}
            for ob in range(NFB):
                for li, l0 in enumerate((0, L - LT)):
                    pss[(ob, li)] = ppool.tile([P, LT], f32, tag=f"ps{ob}{li}",
                                               name=f"ps{ob}{li}_{b}")

            def mm(ob, li, ic, k):
                l0 = (0, L - LT)[li]
                nc.tensor.matmul(
                    pss[(ob, li)][:],
                    w_sb[k][ic][:, ob * P:(ob + 1) * P],
                    xh[:, ic * S + l0 + k: ic * S + l0 + k + LT],
                    start=(ic == 0 and k == 0),
                    stop=(ic == NFB - 1 and k == K - 1),
                )

            if b == 0:
                for ob in range(NFB):
                    for li in range(2):
                        for ic in range(NFB):
                            for k in range(K):
                                mm(ob, li, ic, k)
            else:
                for ic in range(NFB):
                    for k in range(K):
                        for ob in range(NFB):
                            for li in range(2):
                                mm(ob, li, ic, k)

            # tau (fp16, both obs in one [128, 2048] tile at cols ob*1024)
            tau = tpool.tile([P, W2], f16, tag="tau")
            for ob in range(NFB):
                for li, l0 in enumerate((0, L - LT)):
                    nc.scalar.activation(
                        tau[:, ob * S + l0: ob * S + l0 + LT],
                        pss[(ob, li)][:], SIG,
                        bias=bias_sb[:, ob:ob + 1], scale=1.0)

            t2 = tpool.tile([P, W2], f16, tag="t2")
            t4 = tpool.tile([P, W2], f16, tag="t4")
            r = opool.tile([P, W2], f16, tag="r")
            u_all = qpool.tile([P, 4 * W2], f16, tag="u4")
            q_all = qpool.tile([P, 4 * W2], f16, tag="q4")
            mh = qpool.tile([P, 2 * W2], f16, tag="mh")
            hh = qpool.tile([P, 2 * W2], f16, tag="hh")
            m1 = qpool.tile([P, W2], f16, tag="m1")
            nh = qpool.tile([P, W2], f16, tag="nh")
            oh = opool.tile([P, W2], f16, tag="oh")

            # Elementwise chain, fp16 2x-mode on DVE, window-batched:
            #   u_all[j] = tau * x_{2j}           (4 windows in one op)
            #   q_all[j] = u_all[j] + x_{2j+1}
            #   mh = [q0, q2] * T2  -> [m0, h1]
            #   hh = mh + [q1, q3]  -> [h0, h2]
            #   m1 = h0 * T4;  N = m1 + h2;  out = N * r
            # r = 1/denominator via the degree-4 poly custom op (DVE, 1x),
            # issued first: it only needs tau and runs while ACT does T2/T4.
            # Batch 0 runs the chain once per feature block (fb arg slices
            # the c dim) so DVE starts as soon as fb0's sigmoid lands.
            def chain(fb):
                cs = slice(None) if fb is None else slice(fb, fb + 1)
                fl = slice(None) if fb is None else slice(fb * S, (fb + 1) * S)
                nc.vector._custom_dve(
                    RPOLY, out=r[:, fl], in0=tau[:, fl], in1=c4_sb[:],
                    s0=RP_C1, s1=RP_C2, imm2=RP_C3)
                nc.scalar.activation(t2[:, fl], tau[:, fl], SQU)
                nc.scalar.activation(t4[:, fl], t2[:, fl], SQU)
                nc.vector.tensor_tensor(view4(u_all)[:, :, cs],
                                        repview(tau, 4)[:, :, cs],
                                        winview(xh)[:, :, cs], TT.mult)
                nc.vector.tensor_tensor(view4(q_all)[:, :, cs],
                                        view4(u_all)[:, :, cs],
                                        winview(xo)[:, :, cs], TT.add)
                nc.vector.tensor_tensor(view2(mh)[:, :, cs],
                                        qsel(q_all, 0)[:, :, cs],
                                        repview(t2, 2)[:, :, cs], TT.mult)
                nc.vector.tensor_tensor(view2(hh)[:, :, cs],
                                        view2(mh)[:, :, cs],
                                        qsel(q_all, 1)[:, :, cs], TT.add)
                nc.vector.tensor_tensor(pair(m1)[:, cs], jslice(hh, 0)[:, cs],
                                        pair(t4)[:, cs], TT.mult)
                nc.vector.tensor_tensor(pair(nh)[:, cs], pair(m1)[:, cs],
                                        jslice(hh, 1)[:, cs], TT.add)
                nc.vector.tensor_tensor(pair(oh)[:, cs], pair(nh)[:, cs],
                                        pair(r)[:, cs], TT.mult)

            if b == 0:
                chain(0)
                chain(1)
            else:
                chain(None)
            for ob in range(NFB):
                nc.sync.dma_start(yt_d[b, ob], oh[:, ob * S: ob * S + L])

    nc.compile()
    return nc


_NC = None


def _get_nc():
    global _NC
    if _NC is None:
        _NC = build_module()
    return _NC


def prep_inputs(x, conv_w, conv_b):
    xt = np.ascontiguousarray(
        np.asarray(x).transpose(0, 2, 1)).astype(np.float16)
    xh = xt.reshape(B, NFB * P, S).reshape(B, NFB, P, S)
    # interleave the two feature blocks side by side: [B, P, NFB*S]
    xh = np.ascontiguousarray(xh.transpose(0, 2, 1, 3)).reshape(B, P, W2)
    xo = np.empty_like(xh)
    xo[:, :, :W2 - 1] = xh[:, :, 1:]
    xo[:, :, W2 - 1] = 0
    wt = np.ascontiguousarray(
        np.asarray(conv_w).transpose(2, 1, 0)).astype(np.float16)
    wt = wt.reshape(K, NFB, P, F)
    cb = np.ascontiguousarray(conv_b, dtype=np.float32).reshape(F, 1)
    return xh, xo, wt, cb


def make_in_maps(x, conv_w, conv_b):
    xh, xo, wt, cb = prep_inputs(x, conv_w, conv_b)
    return [
        {"xh": xh[c * BPC:(c + 1) * BPC], "xo": xo[c * BPC:(c + 1) * BPC],
         "wt": wt, "cb": cb}
        for c in range(NCORES)
    ]


def gather_output(results):
    out = np.empty((B, L, F), np.float32)
    for c in range(NCORES):
        yt = results[c]["yt"].astype(np.float32)  # [BPC, NFB, P, L]
        out[c * BPC:(c + 1) * BPC] = (
            yt.transpose(0, 3, 1, 2).reshape(BPC, L, F))
    return out


def kernel(x, conv_w, conv_b):
    nc = _get_nc()
    in_maps = make_in_maps(x, conv_w, conv_b)
    res = run_bass_kernel_spmd(nc, in_maps, core_ids=list(range(NCORES)))
    return gather_output(res.results)


# revision 13
# speedup vs baseline: 1.2169x; 1.0013x over previous
"""Trainium2 Bass kernel for nn_ConvFilter (geometric-series conv filter).

Math (per batch b, output position l, feature f):
    t[o,l]  = sum_{i,k} conv_w[o,i,k] * x[l+k,i]          (valid conv, L=S-K+1)
    tau     = sigmoid(t + bias)
    out     = (sum_i tau^(7-i) * x[l+i,f]) / (sum_i tau^i)

Implementation (v2 — fp16 end-to-end on device):
  * host pre-transposes x to [feature, seq] fp16 (aligned + 1-shifted copy
    so every even/odd window stays 4-byte aligned for the DVE 2x mode);
    weights converted to fp16 on host. The SAME fp16 x tiles feed both the
    PE conv (fp16 matmul, full rate) and the DVE elementwise chain.
  * conv: 16 accumulating fp16 matmuls per 512-wide l-tile; two overlapping
    l-tiles (0 and L-512) per output-feature block; PSUM fp32.
  * tau = sigmoid(psum+bias) on ACT (fp16 out); T2 = tau^2, T4 = T2^2 on ACT.
  * numerator via Estrin in fp16 2x mode on DVE:
        q_j = tau*x_{2j} + x_{2j+1}
        N   = (q0*T2 + q1)*T4 + (q2*T2 + q3)
  * 1/denominator as ONE custom DVE op: r = p5(tau) where p5 is the degree-5
    relative-minimax fit of 1/((1+t)(1+t^2)(1+t^4)) on [0,1] with p(0)=1,
    p'(0)=-1 pinned (max rel err 1.4e-3). out = N * r in fp16.
  * output DMA'd as fp16; host converts to fp32.
  * data-parallel over batch: 8 batches/core on 8 cores, weights replicated.
"""

import numpy as np
from contextlib import ExitStack

import concourse.bass as bass
import concourse.tile as tile
from concourse import bacc, mybir
from concourse.bass_utils import run_bass_kernel_spmd
from concourse import dve_ops
from concourse.dve_ops import DveOp
from concourse.dve_spec import (
    Spec, Src0, C0, C1, C2, C3, One, sq, lower, _has_src1, _spill_c3_to_src1,
)
from concourse.dve_uop import DveOpSpec

B, S, F, K = 64, 1024, 256, 8
L = S - K + 1  # 1017
NCORES = 8
BPC = B // NCORES
P = 128
NFB = F // P  # 2 feature blocks
LT = 512      # matmul l-tile width (one PSUM bank)
LE = L + 1    # even fp16 elementwise width (DVE 2x mode needs even counts)
W2 = NFB * S  # 2048: both feature blocks side by side

# degree-4 relative-minimax fit of 1/((1+t)(1+t^2)(1+t^4)) on [0,1] with
# p(0)=1 pinned; max rel err 2.66e-3.
RP_C1 = -0.9619214
RP_C2 = -0.1795749
RP_C3 = 0.156929
RP_C4 = 0.1098995


def _register_op(name, spec, subdim=False):
    for existing in dve_ops.OPS:
        if existing.name == name:
            return existing
    shas = {}
    for ver in ("v3", "v4"):
        tmp = DveOpSpec(name=name, opcode=0, uops=lower(spec, ver=ver),
                        rd1_en=_has_src1(spec))
        shas[ver] = tmp.sha(ver)
    op = DveOp(name, spec, subdim=subdim, uops_sha=shas)
    dve_ops.OPS.append(op)
    dve_ops.CUSTOM_DVE_SPECS[name] = spec
    dve_ops._SUB_OPCODE_FOR_NAME[name] = (
        dve_ops._CUSTOM_DVE_ROW_BASE + len(dve_ops.OPS) - 1
    )
    assert dve_ops._SUB_OPCODE_FOR_NAME[name] < 0x20
    return op


def _get_rpoly_op():
    # r = 1 + t(c1 + t(c2 + t(c3 + c4 t)))  — Horner, c4 spilled to Src1.
    t = Src0
    h = C3 * t
    h = C2 + h
    h = h * t
    h = C1 + h
    h = h * t
    h = C0 + h
    h = h * t
    body = One + h

    def _ref(in0, in1, s0, s1, imm2):
        c4 = in1.reshape(in1.shape[0], -1)[:, :1]
        tt = in0.astype(np.float32)
        return (1.0 + tt * (s0 + tt * (s1 + tt * (imm2 + c4 * tt)))
                ).astype(np.float32)

    spec = Spec(body=_spill_c3_to_src1(body), reference=_ref)
    return _register_op("ANT_CF_RPOLY4", spec)


def build_module():
    RPOLY = _get_rpoly_op()
    f32 = mybir.dt.float32
    f16 = mybir.dt.float16
    TT = mybir.AluOpType
    SIG = mybir.ActivationFunctionType.Sigmoid
    SQU = mybir.ActivationFunctionType.Square

    nc = bacc.Bacc("TRN2", target_bir_lowering=False, debug=False,
                   enable_asserts=False, num_devices=NCORES)

    xh_d = nc.dram_tensor("xh", [BPC, P, W2], f16, kind="ExternalInput").ap()
    xo_d = nc.dram_tensor("xo", [BPC, P, W2], f16, kind="ExternalInput").ap()
    wt_d = nc.dram_tensor("wt", [K, NFB, P, F], f16, kind="ExternalInput").ap()
    cb_d = nc.dram_tensor("cb", [F, 1], f32, kind="ExternalInput").ap()
    yt_d = nc.dram_tensor("yt", [BPC, NFB, P, L], f16, kind="ExternalOutput").ap()

    with tile.TileContext(nc) as tc, ExitStack() as ctx:
        wpool = ctx.enter_context(tc.tile_pool(name="w", bufs=1))
        xpool = ctx.enter_context(tc.tile_pool(name="x", bufs=2))
        tpool = ctx.enter_context(tc.tile_pool(name="t", bufs=2))
        qpool = ctx.enter_context(tc.tile_pool(name="q", bufs=2))
        opool = ctx.enter_context(tc.tile_pool(name="o", bufs=2))
        ppool = ctx.enter_context(tc.tile_pool(name="p", bufs=2, space="PSUM"))

        def load_x(b):
            # x^T fp16, both feature blocks side by side: [128, 2048]
            # xh: aligned copy; xo: 1-left-shifted copy (odd windows).
            # xh split per feature block so the ic=0 matmuls start as soon
            # as the first half lands.
            xh = xpool.tile([P, W2], f16, tag="xh")
            nc.sync.dma_start(xh[:, :S], xh_d[b][:, :S])
            nc.sync.dma_start(xh[:, S:], xh_d[b][:, S:])
            xo = xpool.tile([P, W2], f16, tag="xo")
            nc.sync.dma_start(xo[:], xo_d[b])
            return xh, xo

        # Prologue DMA order matters: batch-0's first matmuls need only
        # xh fb0 + the ic=0 weights, so those go first; xo isn't read until
        # the DVE chain (~25us in) and goes last.
        xh0 = xpool.tile([P, W2], f16, tag="xh")
        nc.sync.dma_start(xh0[:, :S], xh_d[0][:, :S])
        w_sb = [[None] * NFB for _ in range(K)]
        for ic in range(NFB):
            for k in range(K):
                t = wpool.tile([P, F], f16, tag=f"w{k}{ic}")
                w_sb[k][ic] = t
        for k in range(K):
            nc.sync.dma_start(w_sb[k][0][:], wt_d[k, 0])
        nc.sync.dma_start(xh0[:, S:], xh_d[0][:, S:])
        for k in range(K):
            nc.sync.dma_start(w_sb[k][1][:], wt_d[k, 1])
        xo0 = xpool.tile([P, W2], f16, tag="xo")
        nc.sync.dma_start(xo0[:], xo_d[0])
        x0 = (xh0, xo0)
        bias_sb = wpool.tile([P, NFB], f32, tag="bias")
        nc.sync.dma_start(
            bias_sb[:], cb_d.rearrange("(ob p) one -> p (ob one)", p=P))
        c4_sb = wpool.tile([P, 1], f32, tag="c4")
        nc.gpsimd.memset(c4_sb[:], RP_C4)

        def pair(t, off=0):
            return t[:].rearrange("p (c n) -> p c n", c=2)[:, :, off:off + LE]

        def winview(t):
            # overlapping even windows of a [P, W2] tile:
            # [P, j:4 (stride 2), c:2 (stride S), n:LE (stride 1)]
            ap = t[:].rearrange("p (j c n) -> p j c n", j=4, c=2)
            raw = ap.ap
            raw[1] = [2, 4]
            raw[2] = [S, 2]
            raw[3] = [1, LE]
            ap.ap = raw
            return ap

        def repview(t, nj):
            # [P, W2] tile repeated nj times along a stride-0 j dim
            ap = t[:].rearrange("p (j c n) -> p j c n", j=nj, c=2)
            raw = ap.ap
            raw[1] = [0, nj]
            raw[2] = [S, 2]
            raw[3] = [1, LE]
            ap.ap = raw
            return ap

        def view4(t):   # natural [P, 4, 2, LE] view of a [P, 4*W2] tile
            return t[:].rearrange("p (j c n) -> p j c n", j=4, c=2)[:, :, :, :LE]

        def qsel(t, start):  # j in {start, start+2} of a [P, 4*W2] tile
            return t[:].rearrange(
                "p (a j c n) -> p a j c n", a=2, j=2, c=2
            )[:, :, start, :, :LE]

        def view2(t):   # natural [P, 2, 2, LE] view of a [P, 2*W2] tile
            return t[:].rearrange("p (j c n) -> p j c n", j=2, c=2)[:, :, :, :LE]

        def jslice(t, j):  # single j of a [P, 2*W2] tile -> [P, 2, LE]
            return t[:].rearrange("p (j c n) -> p j c n", j=2, c=2)[:, j, :, :LE]

        for b in range(BPC):
            xh, xo = x0 if b == 0 else load_x(b)

            # conv -> 4 PSUM tiles per batch (2 out-blocks x 2 l-tiles).
            # Batch 0 runs group-major so fb0's tau is ready ~10us earlier;
            # steady state runs weight-major (each LDWEIGHTS feeds 4 MMs).
            pss = {}
            for ob in range(NFB):
                for li, l0 in enumerate((0, L - LT)):
                    pss[(ob, li)] = ppool.tile([P, LT], f32, tag=f"ps{ob}{li}",
                                               name=f"ps{ob}{li}_{b}")

            def mm(ob, li, ic, k):
                l0 = (0, L - LT)[li]
                nc.tensor.matmul(
                    pss[(ob, li)][:],
                    w_sb[k][ic][:, ob * P:(ob + 1) * P],
                    xh[:, ic * S + l0 + k: ic * S + l0 + k + LT],
                    start=(ic == 0 and k == 0),
                    stop=(ic == NFB - 1 and k == K - 1),
                )

            if b == 0:
                for ob in range(NFB):
                    for li in range(2):
                        for ic in range(NFB):
                            for k in range(K):
                                mm(ob, li, ic, k)
            else:
                for ic in range(NFB):
                    for k in range(K):
                        for ob in range(NFB):
                            for li in range(2):
                                mm(ob, li, ic, k)

            # tau (fp16, both obs in one [128, 2048] tile at cols ob*1024)
            tau = tpool.tile([P, W2], f16, tag="tau")
            for ob in range(NFB):
                for li, l0 in enumerate((0, L - LT)):
                    nc.scalar.activation(
                        tau[:, ob * S + l0: ob * S + l0 + LT],
                        pss[(ob, li)][:], SIG,
                        bias=bias_sb[:, ob:ob + 1], scale=1.0)

            t2 = tpool.tile([P, W2], f16, tag="t2")
            t4 = tpool.tile([P, W2], f16, tag="t4")
            r = opool.tile([P, W2], f16, tag="r")
            u_all = qpool.tile([P, 4 * W2], f16, tag="u4")
            q_all = qpool.tile([P, 4 * W2], f16, tag="q4")
            mh = qpool.tile([P, 2 * W2], f16, tag="mh")
            hh = qpool.tile([P, 2 * W2], f16, tag="hh")
            m1 = qpool.tile([P, W2], f16, tag="m1")
            nh = qpool.tile([P, W2], f16, tag="nh")
            oh = opool.tile([P, W2], f16, tag="oh")

            # Elementwise chain, fp16 2x-mode on DVE, window-batched:
            #   u_all[j] = tau * x_{2j}           (4 windows in one op)
            #   q_all[j] = u_all[j] + x_{2j+1}
            #   mh = [q0, q2] * T2  -> [m0, h1]
            #   hh = mh + [q1, q3]  -> [h0, h2]
            #   m1 = h0 * T4;  N = m1 + h2;  out = N * r
            # r = 1/denominator via the degree-4 poly custom op (DVE, 1x),
            # issued first: it only needs tau and runs while ACT does T2/T4.
            # Batch 0 runs the chain once per feature block (fb arg slices
            # the c dim) so DVE starts as soon as fb0's sigmoid lands.
            def chain(fb):
                cs = slice(None) if fb is None else slice(fb, fb + 1)
                fl = slice(None) if fb is None else slice(fb * S, (fb + 1) * S)
                nc.vector._custom_dve(
                    RPOLY, out=r[:, fl], in0=tau[:, fl], in1=c4_sb[:],
                    s0=RP_C1, s1=RP_C2, imm2=RP_C3)
                nc.scalar.activation(t2[:, fl], tau[:, fl], SQU)
                nc.scalar.activation(t4[:, fl], t2[:, fl], SQU)
                nc.vector.tensor_tensor(view4(u_all)[:, :, cs],
                                        repview(tau, 4)[:, :, cs],
                                        winview(xh)[:, :, cs], TT.mult)
                nc.vector.tensor_tensor(view4(q_all)[:, :, cs],
                                        view4(u_all)[:, :, cs],
                                        winview(xo)[:, :, cs], TT.add)
                nc.vector.tensor_tensor(view2(mh)[:, :, cs],
                                        qsel(q_all, 0)[:, :, cs],
                                        repview(t2, 2)[:, :, cs], TT.mult)
                nc.vector.tensor_tensor(view2(hh)[:, :, cs],
                                        view2(mh)[:, :, cs],
                                        qsel(q_all, 1)[:, :, cs], TT.add)
                nc.vector.tensor_tensor(pair(m1)[:, cs], jslice(hh, 0)[:, cs],
                                        pair(t4)[:, cs], TT.mult)
                nc.vector.tensor_tensor(pair(nh)[:, cs], pair(m1)[:, cs],
                                        jslice(hh, 1)[:, cs], TT.add)
                nc.vector.tensor_tensor(pair(oh)[:, cs], pair(nh)[:, cs],
                                        pair(r)[:, cs], TT.mult)

            if b == 0 or b == BPC - 1:
                # split per feature block: batch 0 starts the moment fb0's
                # sigmoid lands; the last batch's fb0 out-DMA overlaps fb1.
                for fb in range(NFB):
                    chain(fb)
                    nc.sync.dma_start(yt_d[b, fb], oh[:, fb * S: fb * S + L])
            else:
                chain(None)
                for ob in range(NFB):
                    nc.sync.dma_start(yt_d[b, ob], oh[:, ob * S: ob * S + L])

    nc.compile()
    return nc


_NC = None


def _get_nc():
    global _NC
    if _NC is None:
        _NC = build_module()
    return _NC


def prep_inputs(x, conv_w, conv_b):
    xt = np.ascontiguousarray(
        np.asarray(x).transpose(0, 2, 1)).astype(np.float16)
    xh = xt.reshape(B, NFB * P, S).reshape(B, NFB, P, S)
    # interleave the two feature blocks side by side: [B, P, NFB*S]
    xh = np.ascontiguousarray(xh.transpose(0, 2, 1, 3)).reshape(B, P, W2)
    xo = np.empty_like(xh)
    xo[:, :, :W2 - 1] = xh[:, :, 1:]
    xo[:, :, W2 - 1] = 0
    wt = np.ascontiguousarray(
        np.asarray(conv_w).transpose(2, 1, 0)).astype(np.float16)
    wt = wt.reshape(K, NFB, P, F)
    cb = np.ascontiguousarray(conv_b, dtype=np.float32).reshape(F, 1)
    return xh, xo, wt, cb


def make_in_maps(x, conv_w, conv_b):
    xh, xo, wt, cb = prep_inputs(x, conv_w, conv_b)
    return [
        {"xh": xh[c * BPC:(c + 1) * BPC], "xo": xo[c * BPC:(c + 1) * BPC],
         "wt": wt, "cb": cb}
        for c in range(NCORES)
    ]


def gather_output(results):
    out = np.empty((B, L, F), np.float32)
    for c in range(NCORES):
        yt = results[c]["yt"].astype(np.float32)  # [BPC, NFB, P, L]
        out[c * BPC:(c + 1) * BPC] = (
            yt.transpose(0, 3, 1, 2).reshape(BPC, L, F))
    return out


def kernel(x, conv_w, conv_b):
    nc = _get_nc()
    in_maps = make_in_maps(x, conv_w, conv_b)
    res = run_bass_kernel_spmd(nc, in_maps, core_ids=list(range(NCORES)))
    return gather_output(res.results)
